# revision 14
# baseline (speedup 1.0000x reference)
"""Multi-head attention + residual + LayerNorm on 8 Trainium2 NeuronCores.

Reference computation (B=2, S=2048, D=1024, H=16, HD=64):
    q,k,v = split_heads(x@Wq+bq), ...       # [B,H,S,HD]
    attn  = softmax(q k^T / sqrt(HD))
    out   = (attn v) merged -> [B,S,D] @ Wp + bp
    y     = LayerNorm(x + out) * gamma + beta

Sharding: 8 cores = 2 batches x 4 head-groups (tensor parallel over the
16 heads).  Each core computes Q/K/V for its own 4 heads over the full
2048-token sequence (no K/V exchange needed), runs attention + the
partial out-projection for those heads, and the partials are
ReduceScattered (bf16, one op per 512-query block) across the 4 cores
of the batch.  Core g of a group owns query rows {qb*512 + g*128 ..
+128} for qb in 0..3 and finishes them with residual + LayerNorm.

Numerics: QKV projections and scores in bf16 (f32 PSUM accum); softmax
exp runs on the scalar engine directly from PSUM into fp8-e4m3
attention weights (with a uniform exp(-2) bias that cancels in the
normalization); V, the normalized attention output, and Wp are fp8 so
attn*V and the out-projection run in DoubleRow (dual-pumped fp8) mode.
Residual + LayerNorm in f32.  Validated vs the f32 reference:
rel err ~2e-3 (tolerance 2e-2).
"""

import os

import ml_dtypes
import numpy as np

import concourse.bacc as bacc
import concourse.bass as bass
import concourse.tile as tile
from concourse import mybir
from concourse.bass_utils import run_bass_kernel_spmd

B, S, D, H, HD = 2, 2048, 1024, 16, 64
EPS = 1e-5
NCORES = 8
GROUPS = [[0, 1, 2, 3], [4, 5, 6, 7]]
BF = mybir.dt.bfloat16
F8 = mybir.dt.float8e4
F32 = mybir.dt.float32
Act = mybir.ActivationFunctionType
Alu = mybir.AluOpType
DR = mybir.MatmulPerfMode.DoubleRow

QB = 4          # query blocks of 512
NH = 4          # heads per core
OSC = 32.0      # fp8 scale for normalized attn output
WSC = 16.0      # fp8 scale for Wp


def build_program():
    nc = bacc.Bacc("TRN2", target_bir_lowering=False, debug=False,
                   num_devices=NCORES)

    # ---- I/O ----
    xt_d = nc.dram_tensor("xt", [8, 128, S], BF, kind="ExternalInput")
    wq_d = nc.dram_tensor("wq", [8, 128, 256], BF, kind="ExternalInput")
    wk_d = nc.dram_tensor("wk", [8, 128, 256], BF, kind="ExternalInput")
    wv_d = nc.dram_tensor("wv", [8, 128, 256], BF, kind="ExternalInput")
    wp_d = nc.dram_tensor("wp", [64, 2, 2, D], F8, kind="ExternalInput")
    bq_d = nc.dram_tensor("bq", [2, 128], F32, kind="ExternalInput")
    bk_d = nc.dram_tensor("bk", [2, 128], F32, kind="ExternalInput")   # pre-scaled x8
    bv_d = nc.dram_tensor("bv", [256], F32, kind="ExternalInput")
    xres_d = nc.dram_tensor("xres", [QB, 128, D], F32, kind="ExternalInput")  # x rows + bp
    gamma_d = nc.dram_tensor("gamma", [D], F32, kind="ExternalInput")
    beta_d = nc.dram_tensor("beta", [D], F32, kind="ExternalInput")
    y_d = nc.dram_tensor("y", [QB, 128, D], F32, kind="ExternalOutput")
    # DRAM scratch for the per-unit reciprocal broadcast round-trip
    rdram = nc.dram_tensor("rdram", [16, 512], BF, kind="Internal")
    sdram = nc.dram_tensor("sdram", [16, 512], F32, kind="Internal")

    def bcast_ap(dram_t, n, parts=128):
        return bass.AP(tensor=dram_t, offset=0, ap=[[0, parts], [1, n]])

    def rd_ap(u, ap):
        return bass.AP(tensor=rdram, offset=u * 512, ap=ap)

    def sd_ap(u, ap):
        return bass.AP(tensor=sdram, offset=u * 512, ap=ap)

    with tile.TileContext(nc) as tc:
        with (
            tc.tile_pool(name="persist", bufs=1) as persist,
            tc.tile_pool(name="dram", bufs=1, space="DRAM") as dram,
        ):
            # persistent tiles
            xt_sb = persist.tile([128, 8, S], BF)            # x^T, D-chunk major
            wq_sb = persist.tile([128, 8, 256], BF)
            wk_sb = persist.tile([128, 8, 256], BF)
            wv_sb = persist.tile([128, 8, 256], BF)
            wp_sb = persist.tile([64, 2, 2, D], F8)
            kt_sb = [persist.tile([128, S], BF, name=f"kt{p}") for p in range(2)]
            qt_sb = [persist.tile([128, S], BF, name=f"qt{p}") for p in range(2)]
            vone = persist.tile([128, NH, 16, 80], F8)       # V | ones | pad (16B-aligned sub-stride)
            outT = persist.tile([64, NH, S], F8)             # normalized o^T * 32
            xres_sb = persist.tile([128, QB, D], F32)
            bq_sb = persist.tile([128, 2], F32)
            bk_sb = persist.tile([128, 2], F32)
            bv_bc = persist.tile([128, 4, 64], F32)
            gamma_bc = persist.tile([128, D], F32)
            beta_bc = persist.tile([128, D], F32)
            eps_sb = persist.tile([128, 1], F32)
            nb2_sb = persist.tile([128, 1], F32)     # exp bias: -2.0

            # DRAM scratch: ReduceScatter in/out per query block
            rs_in = [dram.tile([4, 128 * D], BF, name=f"rsi{i}") for i in range(QB)]
            rs_out = [dram.tile([128 * D], BF, name=f"rso{i}") for i in range(QB)]
            dumb_in = dram.tile([1, 128], BF, name="dumb_in")
            dumb_out = dram.tile([4, 128], BF, name="dumb_out")
            nc.gpsimd.collective_compute(
                "AllGather", Alu.bypass, replica_groups=GROUPS,
                ins=[dumb_in[:].opt()], outs=[dumb_out[:].opt()])

            # loads, in need-order (x^T split per D-chunk so QK can start
            # as soon as the last chunk lands)
            for c in range(8):
                nc.sync.dma_start(xt_sb[:, c, :], xt_d[c])
            nc.sync.dma_start(wk_sb[:], wk_d.ap().rearrange("c p m -> p c m"))
            nc.sync.dma_start(wq_sb[:], wq_d.ap().rearrange("c p m -> p c m"))
            nc.sync.dma_start(bk_sb[:], bk_d.ap().rearrange("r p -> p r"))
            nc.sync.dma_start(bq_sb[:], bq_d.ap().rearrange("r p -> p r"))
            nc.sync.dma_start(wv_sb[:], wv_d.ap().rearrange("c p m -> p c m"))
            nc.sync.dma_start(bv_bc[:], bcast_ap(bv_d, 256))
            nc.sync.dma_start(wp_sb[:], wp_d.ap())
            nc.sync.dma_start(xres_sb[:], xres_d.ap().rearrange("q p d -> p q d"))
            nc.sync.dma_start(gamma_bc[:], bcast_ap(gamma_d, D))
            nc.sync.dma_start(beta_bc[:], bcast_ap(beta_d, D))
            nc.vector.memset(eps_sb[:], EPS)
            nc.vector.memset(nb2_sb[:], -2.0)
            for h in range(NH):
                nc.vector.memset(vone[:, h, :, 64:80], 0.0)
                nc.vector.memset(vone[:, h, :, 64:65], 1.0)

            with (
                tc.tile_pool(name="work", bufs=2) as work,
                tc.tile_pool(name="expp", bufs=24) as expp,
                tc.tile_pool(name="small", bufs=4) as small,
                tc.tile_pool(name="ps_sc", bufs=3, space="PSUM") as ps_sc,
                tc.tile_pool(name="ps_o", bufs=2, space="PSUM") as ps_o,
            ):
                # ---------- phase 1 pieces ----------
                def qk_pair(pair):
                    # K^T then Q^T for heads {2*pair, 2*pair+1}: [128, S]
                    for typ in range(2):    # 0 = K, 1 = Q
                        w_sb = (wk_sb, wq_sb)[typ]
                        dst = (kt_sb, qt_sb)[typ][pair]
                        b_sb = (bk_sb, bq_sb)[typ]
                        for nck in range(4):
                            ps = ps_sc.tile([128, 2, 512], F32, tag="sc",
                                            name=f"qk{pair}{typ}{nck}")
                            for kc in range(8):
                                nc.tensor.matmul(
                                    ps[:, 0, :],
                                    w_sb[:, kc, pair * 128:(pair + 1) * 128],
                                    xt_sb[:, kc, nck * 512:(nck + 1) * 512],
                                    start=(kc == 0), stop=(kc == 7))
                            # (psum + bias) * scale  (K pre-scaled by 1/8)
                            with nc.allow_low_precision("K/Q in bf16"):
                                nc.vector.tensor_scalar(
                                    out=dst[:, nck * 512:(nck + 1) * 512],
                                    in0=ps[:, 0, :],
                                    scalar1=b_sb[:, pair:pair + 1],
                                    scalar2=0.125 if typ == 0 else 1.0,
                                    op0=Alu.add, op1=Alu.mult)

                def v_proj():
                    for tcn in range(16):
                        ps = ps_sc.tile([128, 2, 512], F32, tag="sc",
                                        name=f"v{tcn}")
                        for kc in range(8):
                            nc.tensor.matmul(
                                ps[:, 0, 0:256],
                                xt_sb[:, kc, tcn * 128:(tcn + 1) * 128],
                                wv_sb[:, kc, :],
                                start=(kc == 0), stop=(kc == 7))
                        with nc.allow_low_precision("attn V in fp8"):
                            nc.vector.tensor_add(
                                vone[:, :, tcn, 0:64],
                                ps[:, 0, 0:256].rearrange("p (h d) -> p h d", h=4),
                                bv_bc[:])

                # ---------- phase 2 pieces ----------
                def scores_exp(h, qb):
                    pair, half = h // 2, h % 2
                    r0 = half * 64
                    exp_ts = []
                    for j in range(8):      # key-chunk pairs of 256
                        ps = ps_sc.tile([128, 2, 512], F32, tag="sc",
                                        name=f"s{h}{qb}{j}")
                        for s in range(2):
                            kc = 2 * j + s
                            nc.tensor.matmul(
                                ps[:, s, :],
                                kt_sb[pair][r0:r0 + 64, kc * 128:(kc + 1) * 128],
                                qt_sb[pair][r0:r0 + 64, qb * 512:(qb + 1) * 512],
                                start=True, stop=True)
                        et = expp.tile([128, 2, 512], F8, tag="exp",
                                       name=f"e{h}{qb}{j}")
                        nc.scalar.activation(et[:], ps[:], Act.Exp, bias=nb2_sb[:])
                        exp_ts.append(et)
                    return exp_ts

                def attn_v(h, qb, exp_ts):
                    oT = ps_o.tile([80, 512], F32, tag="o", name=f"o{h}{qb}")
                    for j in range(8):
                        nc.tensor.matmul(
                            oT[:], vone[:, h, 2 * j:2 * j + 2, :], exp_ts[j][:],
                            start=(j == 0), stop=(j == 7), perf_mode=DR)
                    return oT

                def norm_o(h, qb, oT):
                    u = qb * NH + h
                    # sums row 64 -> bf16 -> scatter [128,4] -> recip ->
                    # gather to DRAM -> partition-broadcast read [64,512]
                    s_sb = small.tile([128, 512], F32, tag="ssb", name=f"ss{u}")
                    nc.vector.tensor_copy(s_sb[64:65, :], oT[64:65, :])
                    nc.gpsimd.dma_start(sd_ap(u, [[1, 512]]), s_sb[64:65, :])
                    sc = small.tile([128, 4], F32, tag="sc4", name=f"sc{u}")
                    nc.gpsimd.dma_start(sc[:], sd_ap(u, [[1, 128], [128, 4]]))
                    rcf = small.tile([128, 4], F32, tag="rcf", name=f"rc{u}")
                    nc.vector.reciprocal_approx_fast(out=rcf[:], in_=sc[:])
                    rcb = small.tile([128, 4], BF, tag="rcb", name=f"rb{u}")
                    with nc.allow_low_precision("softmax scale bf16"):
                        nc.vector.tensor_copy(rcb[:], rcf[:])
                    nc.gpsimd.dma_start(rd_ap(u, [[1, 128], [128, 4]]), rcb[:])
                    r_bc = small.tile([64, 512], BF, tag="rbc", name=f"rr{u}")
                    nc.gpsimd.dma_start(r_bc[:], rd_ap(u, [[0, 64], [1, 512]]))
                    with nc.allow_low_precision("attn out fp8"):
                        nc.vector.scalar_tensor_tensor(
                            out=outT[:, h, qb * 512:(qb + 1) * 512],
                            in0=oT[0:64, :], scalar=OSC, in1=r_bc[:],
                            op0=Alu.mult, op1=Alu.mult)

                def unit(h, qb):
                    exp_ts = scores_exp(h, qb)
                    oT = attn_v(h, qb, exp_ts)
                    norm_o(h, qb, oT)

                def proj_rs(qb):
                    for qc in range(4):
                        ps = ps_sc.tile([128, 2, 512], F32, tag="sc",
                                        name=f"pj{qb}{qc}")
                        for ncn in range(2):
                            for c in range(2):
                                nc.tensor.matmul(
                                    ps[:, ncn, :],
                                    outT[:, 2 * c:2 * c + 2,
                                         qb * 512 + qc * 128:qb * 512 + (qc + 1) * 128],
                                    wp_sb[:, :, c, ncn * 512:(ncn + 1) * 512],
                                    start=(c == 0), stop=(c == 1), perf_mode=DR)
                        yb = work.tile([128, 2, 512], BF, tag="yb", name=f"yb{qb}{qc}")
                        with nc.allow_low_precision("partial y bf16"):
                            nc.vector.tensor_scalar_mul(yb[:], ps[:], 1.0 / (OSC * WSC))
                        nc.gpsimd.dma_start(
                            rs_in[qb][qc].rearrange("(p d) -> p d", p=128), yb[:])
                    nc.gpsimd.collective_compute(
                        "ReduceScatter", Alu.add, replica_groups=GROUPS,
                        ins=[rs_in[qb][:].opt()], outs=[rs_out[qb][:].opt()])
                    # load of the scattered result (fires when the CC signals)
                    rsb = work.tile([128, D], BF, tag="rsb", name=f"rsb{qb}")
                    nc.gpsimd.dma_start(
                        rsb[:], rs_out[qb][:].rearrange("(p d) -> p d", p=128))
                    return rsb

                def post(qb, rsb):
                    yt = work.tile([128, D], F32, tag="yt", name=f"yt{qb}")
                    nc.vector.tensor_add(yt[:], rsb[:], xres_sb[:, qb, :])
                    stats = small.tile([128, 2, 6], F32, tag="st", name=f"st{qb}")
                    nc.vector.bn_stats(stats[:, 0, :], yt[:, 0:512])
                    nc.vector.bn_stats(stats[:, 1, :], yt[:, 512:1024])
                    mv = small.tile([128, 2], F32, tag="mv", name=f"mv{qb}")
                    nc.vector.bn_aggr(mv[:], stats[:])
                    # rstd = exp(-0.5 * ln(var + eps)); Ln and Exp share one
                    # activation table set so no table thrash with the exps.
                    lnv = small.tile([128, 1], F32, tag="lnv", name=f"ln{qb}")
                    nc.scalar.activation(lnv[:], mv[:, 1:2], Act.Ln, bias=eps_sb[:])
                    rstd = small.tile([128, 1], F32, tag="rst", name=f"rs{qb}")
                    nc.scalar.activation(rstd[:], lnv[:], Act.Exp, scale=-0.5)
                    # y = ((y - mu) * gamma) * rstd + beta
                    nc.vector.scalar_tensor_tensor(
                        out=yt[:], in0=yt[:], scalar=mv[:, 0:1], in1=gamma_bc[:],
                        op0=Alu.subtract, op1=Alu.mult)
                    nc.vector.scalar_tensor_tensor(
                        out=yt[:], in0=yt[:], scalar=rstd[:], in1=beta_bc[:],
                        op0=Alu.mult, op1=Alu.add)
                    nc.sync.dma_start(y_d[qb], yt[:])

                # ---------- emission: pipeline phase 1 into phase 2 ----------
                qk_pair(0)
                e0 = scores_exp(0, 0)
                v_proj()
                qk_pair(1)
                e1 = scores_exp(1, 0)
                norm_o(0, 0, attn_v(0, 0, e0))
                norm_o(1, 0, attn_v(1, 0, e1))
                unit(2, 0)
                unit(3, 0)
                rsbs = {0: proj_rs(0)}
                for qb in range(1, QB):
                    for h in range(NH):
                        unit(h, qb)
                    rsbs[qb] = proj_rs(qb)
                    post(qb - 1, rsbs[qb - 1])
                post(QB - 1, rsbs[QB - 1])

    nc.compile()
    return nc


_PROGRAM = None


def _get_program():
    global _PROGRAM
    if _PROGRAM is None:
        _PROGRAM = build_program()
    return _PROGRAM


def kernel(**inputs):
    x = np.asarray(inputs["x"], np.float32)
    bf = ml_dtypes.bfloat16
    f8 = ml_dtypes.float8_e4m3
    Wq = np.asarray(inputs["Wq"], np.float32)
    Wk = np.asarray(inputs["Wk"], np.float32)
    Wv = np.asarray(inputs["Wv"], np.float32)
    Wp = np.asarray(inputs["Wp"], np.float32)
    bq = np.asarray(inputs["bq"], np.float32)
    bk = np.asarray(inputs["bk"], np.float32)
    bv = np.asarray(inputs["bv"], np.float32)
    bp = np.asarray(inputs["bp"], np.float32)
    gamma = np.asarray(inputs["gamma"], np.float32)
    beta = np.asarray(inputs["beta"], np.float32)

    xt_b = []
    for b in range(B):
        xt = np.ascontiguousarray(x[b].T).astype(bf)          # [1024, 2048]
        xt_b.append(xt.reshape(8, 128, S))
    in_maps = []
    for c in range(NCORES):
        b, g = c // 4, c % 4
        cs = slice(g * 256, (g + 1) * 256)
        wp8 = (WSC * Wp[cs, :]).astype(f8).reshape(NH, 64, D)  # [head, d, D]
        wp8 = np.ascontiguousarray(
            wp8.reshape(2, 2, 64, D).transpose(2, 1, 0, 3))    # [64, sub, c, D]
        xres = np.empty((QB, 128, D), np.float32)
        for qb in range(QB):
            xres[qb] = x[b, qb * 512 + g * 128: qb * 512 + (g + 1) * 128, :] + bp
        m = {
            "xt": xt_b[b],
            "wq": np.ascontiguousarray(Wq[:, cs]).astype(bf).reshape(8, 128, 256),
            "wk": np.ascontiguousarray(Wk[:, cs]).astype(bf).reshape(8, 128, 256),
            "wv": np.ascontiguousarray(Wv[:, cs]).astype(bf).reshape(8, 128, 256),
            "wp": wp8,
            "bq": np.ascontiguousarray(bq[cs]).reshape(2, 128),
            "bk": np.ascontiguousarray(8.0 * bk[cs]).reshape(2, 128),
            "bv": np.ascontiguousarray(bv[cs]),
            "xres": xres,
            "gamma": gamma,
            "beta": beta,
        }
        in_maps.append(m)

    nc = _get_program()
    trace_dir = os.environ.get("BASS_KERNEL_TRACE_DIR")
    kwargs = {}
    if trace_dir:
        kwargs = {"trace": True, "tmpdir": trace_dir}
    res = run_bass_kernel_spmd(nc, in_maps, core_ids=list(range(NCORES)), **kwargs)

    out = np.empty((B, S, D), np.float32)
    for c in range(NCORES):
        b, g = c // 4, c % 4
        yc = res.results[c]["y"]       # [QB, 128, D]
        for qb in range(QB):
            out[b, qb * 512 + g * 128: qb * 512 + (g + 1) * 128, :] = yc[qb]
    if trace_dir:
        kernel.last_exec_time_ns = res.exec_time_ns
        kernel.last_trace = res.instructions_and_trace
    return out


# revision 20
# speedup vs baseline: 1.6060x; 1.6060x over previous
"""Multi-head attention + residual + LayerNorm on 8 Trainium2 NeuronCores.

Reference computation (B=2, S=2048, D=1024, H=16, HD=64):
    q,k,v = split_heads(x@Wq+bq), ...       # [B,H,S,HD]
    attn  = softmax(q k^T / sqrt(HD))
    out   = (attn v) merged -> [B,S,D] @ Wp + bp
    y     = LayerNorm(x + out) * gamma + beta

Sharding: 8 cores = 2 batches x 4 head-groups (tensor parallel over the
16 heads).  Each core computes Q/K/V for its own 4 heads over the full
2048-token sequence (no K/V exchange needed), runs attention + the
partial out-projection for those heads, and the partials are
ReduceScattered (bf16, one op per 512-query block) across the 4 cores
of the batch.  Core g of a group owns query rows {qb*512 + g*128 ..
+128} for qb in 0..3 and finishes them with residual + LayerNorm.

Numerics: QKV projections and scores in bf16 (f32 PSUM accum); softmax
exp runs on the scalar engine directly from PSUM into fp8-e4m3
attention weights (with a uniform exp(-2) bias that cancels in the
normalization); V, the normalized attention output, and Wp are fp8 so
attn*V and the out-projection run in DoubleRow (dual-pumped fp8) mode.
Residual + LayerNorm in f32.  Validated vs the f32 reference:
rel err ~2e-3 (tolerance 2e-2).
"""

import os

import ml_dtypes
import numpy as np

import concourse.bacc as bacc
import concourse.bass as bass
import concourse.tile as tile
from concourse import mybir
from concourse.bass_utils import run_bass_kernel_spmd

B, S, D, H, HD = 2, 2048, 1024, 16, 64
EPS = 1e-5
NCORES = 8
GROUPS = [[0, 1, 2, 3], [4, 5, 6, 7]]
BF = mybir.dt.bfloat16
F8 = mybir.dt.float8e4
F32 = mybir.dt.float32
Act = mybir.ActivationFunctionType
Alu = mybir.AluOpType
DR = mybir.MatmulPerfMode.DoubleRow

QB = 4          # query blocks of 512
NH = 4          # heads per core
OSC = 32.0      # fp8 scale for normalized attn output
WSC = 16.0      # fp8 scale for Wp


def build_program():
    nc = bacc.Bacc("TRN2", target_bir_lowering=False, debug=False,
                   num_devices=NCORES)

    # ---- I/O ----
    xt_d = nc.dram_tensor("xt", [8, 128, S], BF, kind="ExternalInput")
    wq_d = nc.dram_tensor("wq", [8, 128, 256], BF, kind="ExternalInput")
    wk_d = nc.dram_tensor("wk", [8, 128, 256], BF, kind="ExternalInput")
    wv_d = nc.dram_tensor("wv", [8, 128, 256], BF, kind="ExternalInput")
    wp_d = nc.dram_tensor("wp", [64, 2, 2, D], F8, kind="ExternalInput")
    bq_d = nc.dram_tensor("bq", [2, 128], F32, kind="ExternalInput")
    bk_d = nc.dram_tensor("bk", [2, 128], F32, kind="ExternalInput")   # pre-scaled x8
    bv_d = nc.dram_tensor("bv", [256], F32, kind="ExternalInput")
    xres_d = nc.dram_tensor("xres", [QB, 128, D], F32, kind="ExternalInput")  # x rows + bp
    gamma_d = nc.dram_tensor("gamma", [D], F32, kind="ExternalInput")
    beta_d = nc.dram_tensor("beta", [D], F32, kind="ExternalInput")
    y_d = nc.dram_tensor("y", [QB, 128, D], F32, kind="ExternalOutput")
    dbg1_d = nc.dram_tensor("dbg1", [4, 128 * D], BF, kind="ExternalOutput")
    dbg2_d = nc.dram_tensor("dbg2", [64, NH * S], F8, kind="ExternalOutput")
    # DRAM scratch for the per-unit reciprocal broadcast round-trip
    rdram = nc.dram_tensor("rdram", [16, 512], BF, kind="Internal")
    sdram = nc.dram_tensor("sdram", [16, 512], F32, kind="Internal")

    def bcast_ap(dram_t, n, parts=128):
        return bass.AP(tensor=dram_t, offset=0, ap=[[0, parts], [1, n]])

    def rd_ap(u, ap):
        return bass.AP(tensor=rdram, offset=u * 512, ap=ap)

    def sd_ap(u, ap):
        return bass.AP(tensor=sdram, offset=u * 512, ap=ap)

    with tile.TileContext(nc) as tc:
        with (
            tc.tile_pool(name="persist", bufs=1) as persist,
            tc.tile_pool(name="dram", bufs=1, space="DRAM") as dram,
        ):
            # persistent tiles
            xt_sb = persist.tile([128, 8, S], BF)            # x^T, D-chunk major
            wq_sb = persist.tile([128, 8, 256], BF)
            wk_sb = persist.tile([128, 8, 256], BF)
            wv_sb = persist.tile([128, 8, 256], BF)
            wp_sb = persist.tile([64, 2, 2, D], F8)
            kt_sb = [persist.tile([128, S], BF, name=f"kt{p}") for p in range(2)]
            qt_sb = [persist.tile([128, S], BF, name=f"qt{p}") for p in range(2)]
            vone = persist.tile([128, NH, 16, 80], F8)       # V | ones | pad (16B-aligned sub-stride)
            outT = persist.tile([64, NH, S], F8)             # normalized o^T * 32
            xres_sb = persist.tile([128, QB, D], F32)
            bq_sb = persist.tile([128, 2], F32)
            bk_sb = persist.tile([128, 2], F32)
            bv_bc = persist.tile([128, 4, 64], F32)
            gamma_bc = persist.tile([128, D], F32)
            beta_bc = persist.tile([128, D], F32)
            eps_sb = persist.tile([128, 1], F32)
            nb2_sb = persist.tile([128, 1], F32)     # exp bias: -2.0
            ones_sb = persist.tile([128, 64], BF)    # rank-1 broadcast row

            # DRAM scratch: ReduceScatter in/out per query block
            rs_in = [dram.tile([4, 128 * D], BF, name=f"rsi{i}") for i in range(QB)]
            rs_out = [dram.tile([128 * D], BF, name=f"rso{i}") for i in range(QB)]
            dumb_in = dram.tile([1, 128], BF, name="dumb_in")
            dumb_out = dram.tile([4, 128], BF, name="dumb_out")
            zrow = persist.tile([1, 128], BF)
            nc.vector.memset(zrow[:], 0.0)
            nc.gpsimd.dma_start(dumb_in[:], zrow[:])
            nc.gpsimd.collective_compute(
                "AllGather", Alu.bypass, replica_groups=GROUPS,
                ins=[dumb_in[:].opt()], outs=[dumb_out[:].opt()])

            # loads, in need-order; x^T lands in 512-token slices so the
            # K projection can start after the first ~1MB
            nc.sync.dma_start(wk_sb[:], wk_d.ap().rearrange("c p m -> p c m"))
            nc.sync.dma_start(bk_sb[:], bk_d.ap().rearrange("r p -> p r"))
            nc.sync.dma_start(wq_sb[:], wq_d.ap().rearrange("c p m -> p c m"))
            nc.sync.dma_start(bq_sb[:], bq_d.ap().rearrange("r p -> p r"))
            for s in range(4):
                nc.sync.dma_start(
                    xt_sb[:, :, s * 512:(s + 1) * 512],
                    xt_d.ap().rearrange("c p s -> p c s")[:, :, s * 512:(s + 1) * 512])
            nc.sync.dma_start(wv_sb[:], wv_d.ap().rearrange("c p m -> p c m"))
            nc.sync.dma_start(bv_bc[:], bcast_ap(bv_d, 256))
            nc.sync.dma_start(wp_sb[:], wp_d.ap())
            nc.sync.dma_start(xres_sb[:], xres_d.ap().rearrange("q p d -> p q d"))
            nc.sync.dma_start(gamma_bc[:], bcast_ap(gamma_d, D))
            nc.sync.dma_start(beta_bc[:], bcast_ap(beta_d, D))
            nc.vector.memset(eps_sb[:], EPS)
            nc.vector.memset(ones_sb[:], 1.0)
            nc.vector.memset(nb2_sb[:], -2.0)
            for h in range(NH):
                nc.vector.memset(vone[:, h, :, 64:80], 0.0)
                nc.vector.memset(vone[:, h, :, 64:65], 1.0)

            with (
                tc.tile_pool(name="work", bufs=2) as work,
                tc.tile_pool(name="expp", bufs=24) as expp,
                tc.tile_pool(name="small", bufs=4) as small,
                tc.tile_pool(name="ps_sc", bufs=2, space="PSUM") as ps_sc,
                tc.tile_pool(name="ps_o", bufs=2, space="PSUM") as ps_o,
            ):
                # ---------- phase 1 pieces ----------
                def qk_units(pair, typ, ncs):
                    # K^T / Q^T chunks for heads {2*pair, 2*pair+1}
                    if True:
                        w_sb = (wk_sb, wq_sb)[typ]
                        dst = (kt_sb, qt_sb)[typ][pair]
                        b_sb = (bk_sb, bq_sb)[typ]
                        for nck in ncs:
                            ps = ps_sc.tile([128, 2, 512], F32, tag="sc",
                                            name=f"qk{pair}{typ}{nck}")
                            for kc in range(8):
                                nc.tensor.matmul(
                                    ps[:, 0, :],
                                    w_sb[:, kc, pair * 128:(pair + 1) * 128],
                                    xt_sb[:, kc, nck * 512:(nck + 1) * 512],
                                    start=(kc == 0), stop=(kc == 7))
                            # (psum + bias) * scale  (K pre-scaled by 1/8)
                            with nc.allow_low_precision("K/Q in bf16"):
                                nc.vector.tensor_scalar(
                                    out=dst[:, nck * 512:(nck + 1) * 512],
                                    in0=ps[:, 0, :],
                                    scalar1=b_sb[:, pair:pair + 1],
                                    scalar2=0.125 if typ == 0 else 1.0,
                                    op0=Alu.add, op1=Alu.mult)

                def v_proj():
                    for tcn in range(16):
                        ps = ps_sc.tile([128, 2, 512], F32, tag="sc",
                                        name=f"v{tcn}")
                        for kc in range(8):
                            nc.tensor.matmul(
                                ps[:, 0, 0:256],
                                xt_sb[:, kc, tcn * 128:(tcn + 1) * 128],
                                wv_sb[:, kc, :],
                                start=(kc == 0), stop=(kc == 7))
                        with nc.allow_low_precision("attn V in fp8"):
                            nc.vector.tensor_add(
                                vone[:, :, tcn, 0:64],
                                ps[:, 0, 0:256].rearrange("p (h d) -> p h d", h=4),
                                bv_bc[:])

                # ---------- phase 2 pieces ----------
                def scores_exp(h, qb):
                    pair, half = h // 2, h % 2
                    r0 = half * 64
                    exp_ts = []
                    for j in range(8):      # key-chunk pairs of 256
                        ps = ps_sc.tile([128, 2, 512], F32, tag="sc",
                                        name=f"s{h}{qb}{j}")
                        for s in range(2):
                            kc = 2 * j + s
                            nc.tensor.matmul(
                                ps[:, s, :],
                                kt_sb[pair][r0:r0 + 64, kc * 128:(kc + 1) * 128],
                                qt_sb[pair][r0:r0 + 64, qb * 512:(qb + 1) * 512],
                                start=True, stop=True)
                        et = expp.tile([128, 2, 512], F8, tag="exp",
                                       name=f"e{h}{qb}{j}")
                        nc.scalar.activation(et[:], ps[:], Act.Exp, bias=nb2_sb[:])
                        exp_ts.append(et)
                    return exp_ts

                def attn_v(h, qb, exp_ts):
                    oT = ps_o.tile([80, 512], F32, tag="o", name=f"o{h}{qb}")
                    for j in range(8):
                        nc.tensor.matmul(
                            oT[:], vone[:, h, 2 * j:2 * j + 2, :], exp_ts[j][:],
                            start=(j == 0), stop=(j == 7), perf_mode=DR)
                    return oT

                def norm_o(h, qb, oT):
                    u = qb * NH + h
                    # sums row (partition 64) -> partition 0 via DMA, then
                    # reciprocal, rank-1 ones broadcast on the PE, and a
                    # staged scale+mult to fp8 (baseline-proven recipe)
                    s_sb = small.tile([128, 512], F32, tag="ssb", name=f"ss{u}")
                    nc.vector.tensor_copy(s_sb[64:65, :], oT[64:65, :])
                    s0 = small.tile([1, 512], F32, tag="s0", name=f"s0{u}")
                    nc.gpsimd.dma_start(s0[:], s_sb[64:65, :])
                    rr = small.tile([1, 512], F32, tag="rr", name=f"rw{u}")
                    nc.vector.reciprocal_approx_fast(out=rr[:], in_=s0[:])
                    rb = small.tile([1, 512], BF, tag="rb", name=f"rb{u}")
                    with nc.allow_low_precision("softmax scale bf16"):
                        nc.vector.tensor_copy(rb[:], rr[:])
                    r1 = ps_o.tile([128, 512], F32, tag="pj", name=f"r1{u}")
                    nc.tensor.matmul(r1[0:64, :], ones_sb[0:1, 0:64],
                                     rb[:], start=True, stop=True)
                    oSB = small.tile([64, 512], F32, tag="osb", name=f"ob{u}")
                    nc.vector.tensor_copy(oSB[:], oT[0:64, :])
                    with nc.allow_low_precision("attn out fp8"):
                        nc.vector.scalar_tensor_tensor(
                            out=outT[:, h, qb * 512:(qb + 1) * 512],
                            in0=oSB[:], scalar=OSC, in1=r1[0:64, :],
                            op0=Alu.mult, op1=Alu.mult)

                def unit(h, qb):
                    exp_ts = scores_exp(h, qb)
                    oT = attn_v(h, qb, exp_ts)
                    norm_o(h, qb, oT)

                def proj_rs(qb):
                    for qc in range(4):
                        yb = work.tile([128, 2, 512], BF, tag="yb", name=f"yb{qb}{qc}")
                        for ncn in range(2):
                            ps = ps_o.tile([128, 512], F32, tag="pj",
                                           name=f"pj{qb}{qc}{ncn}")
                            for c in range(2):
                                nc.tensor.matmul(
                                    ps[:],
                                    outT[:, 2 * c:2 * c + 2,
                                         qb * 512 + qc * 128:qb * 512 + (qc + 1) * 128],
                                    wp_sb[:, :, c, ncn * 512:(ncn + 1) * 512],
                                    start=(c == 0), stop=(c == 1), perf_mode=DR)
                            with nc.allow_low_precision("partial y bf16"):
                                nc.vector.tensor_scalar_mul(
                                    yb[:, ncn, :], ps[:], 1.0 / (OSC * WSC))
                        nc.gpsimd.dma_start(
                            rs_in[qb][qc].rearrange("(p d) -> p d", p=128), yb[:])
                    nc.gpsimd.collective_compute(
                        "ReduceScatter", Alu.add, replica_groups=GROUPS,
                        ins=[rs_in[qb][:].opt()], outs=[rs_out[qb][:].opt()])
                    # load of the scattered result (fires when the CC signals)
                    rsb = work.tile([128, D], BF, tag="rsb", name=f"rsb{qb}")
                    nc.gpsimd.dma_start(
                        rsb[:], rs_out[qb][:].rearrange("(p d) -> p d", p=128))
                    return rsb

                def post_pre(qb, rsb):
                    yt = work.tile([128, D], F32, tag="yt", name=f"yt{qb}",
                                   bufs=QB)
                    nc.vector.tensor_add(yt[:], rsb[:], xres_sb[:, qb, :])
                    stats = small.tile([128, 2, 6], F32, tag="st", name=f"st{qb}")
                    nc.vector.bn_stats(stats[:, 0, :], yt[:, 0:512])
                    nc.vector.bn_stats(stats[:, 1, :], yt[:, 512:1024])
                    mv = small.tile([128, 2], F32, tag="mv", name=f"mv{qb}",
                                    bufs=QB)
                    nc.vector.bn_aggr(mv[:], stats[:])
                    return yt, mv

                def post_fin(qb, yt, mv):
                    # ACT Ln/Exp deferred to the tail (under the last RS) so
                    # the activation-table swap never interrupts the exp
                    # stream.  rstd = exp(-0.5 * ln(var + eps)).
                    lnv = small.tile([128, 1], F32, tag="lnv", name=f"ln{qb}")
                    nc.scalar.activation(lnv[:], mv[:, 1:2], Act.Ln, bias=eps_sb[:])
                    rstd = small.tile([128, 1], F32, tag="rst", name=f"rs{qb}")
                    nc.scalar.activation(rstd[:], lnv[:], Act.Exp, scale=-0.5)
                    # y = ((y - mu) * gamma) * rstd + beta
                    nc.vector.scalar_tensor_tensor(
                        out=yt[:], in0=yt[:], scalar=mv[:, 0:1], in1=gamma_bc[:],
                        op0=Alu.subtract, op1=Alu.mult)
                    nc.vector.scalar_tensor_tensor(
                        out=yt[:], in0=yt[:], scalar=rstd[:], in1=beta_bc[:],
                        op0=Alu.mult, op1=Alu.add)
                    nc.sync.dma_start(y_d[qb], yt[:])

                # ---------- emission: software-pipelined ----------
                # proj_rs(qb) is deferred behind the first two units of
                # qb+1 so the PE queue never stalls on the softmax-
                # normalize chains; LN finalization happens in the tail
                # under the last ReduceScatter.
                qk_units(0, 0, [0, 1, 2, 3])     # K pair 0
                qk_units(0, 1, [0])              # Q pair 0, first 512 queries
                e0 = scores_exp(0, 0)
                qk_units(0, 1, [1, 2, 3])
                e1 = scores_exp(1, 0)
                v_proj()
                qk_units(1, 0, [0, 1, 2, 3])
                qk_units(1, 1, [0, 1, 2, 3])
                norm_o(0, 0, attn_v(0, 0, e0))
                norm_o(1, 0, attn_v(1, 0, e1))
                unit(2, 0)
                unit(3, 0)
                nc.sync.dma_start(dbg1_d.ap(), rs_in[0][:])
                rsbs = {}
                posts = {}
                for qb in range(1, QB):
                    unit(0, qb)
                    rsbs[qb - 1] = proj_rs(qb - 1)
                    unit(1, qb)
                    unit(2, qb)
                    if qb >= 2:
                        posts[qb - 2] = post_pre(qb - 2, rsbs[qb - 2])
                    unit(3, qb)
                rsbs[QB - 1] = proj_rs(QB - 1)
                posts[QB - 2] = post_pre(QB - 2, rsbs[QB - 2])
                posts[QB - 1] = post_pre(QB - 1, rsbs[QB - 1])
                for qb in range(QB):
                    post_fin(qb, *posts[qb])
                nc.sync.dma_start(dbg2_d.ap(), outT[:].rearrange("p h s -> p (h s)"))

    nc.compile()
    return nc


_PROGRAM = None


def _get_program():
    global _PROGRAM
    if _PROGRAM is None:
        _PROGRAM = build_program()
    return _PROGRAM


def kernel(**inputs):
    x = np.asarray(inputs["x"], np.float32)
    bf = ml_dtypes.bfloat16
    f8 = ml_dtypes.float8_e4m3
    Wq = np.asarray(inputs["Wq"], np.float32)
    Wk = np.asarray(inputs["Wk"], np.float32)
    Wv = np.asarray(inputs["Wv"], np.float32)
    Wp = np.asarray(inputs["Wp"], np.float32)
    bq = np.asarray(inputs["bq"], np.float32)
    bk = np.asarray(inputs["bk"], np.float32)
    bv = np.asarray(inputs["bv"], np.float32)
    bp = np.asarray(inputs["bp"], np.float32)
    gamma = np.asarray(inputs["gamma"], np.float32)
    beta = np.asarray(inputs["beta"], np.float32)

    xt_b = []
    for b in range(B):
        xt = np.ascontiguousarray(x[b].T).astype(bf)          # [1024, 2048]
        xt_b.append(xt.reshape(8, 128, S))
    in_maps = []
    for c in range(NCORES):
        b, g = c // 4, c % 4
        cs = slice(g * 256, (g + 1) * 256)
        wp8 = (WSC * Wp[cs, :]).astype(f8).reshape(NH, 64, D)  # [head, d, D]
        wp8 = np.ascontiguousarray(
            wp8.reshape(2, 2, 64, D).transpose(2, 1, 0, 3))    # [64, sub, c, D]
        xres = np.empty((QB, 128, D), np.float32)
        for qb in range(QB):
            xres[qb] = x[b, qb * 512 + g * 128: qb * 512 + (g + 1) * 128, :] + bp
        m = {
            "xt": xt_b[b],
            "wq": np.ascontiguousarray(Wq[:, cs]).astype(bf).reshape(8, 128, 256),
            "wk": np.ascontiguousarray(Wk[:, cs]).astype(bf).reshape(8, 128, 256),
            "wv": np.ascontiguousarray(Wv[:, cs]).astype(bf).reshape(8, 128, 256),
            "wp": wp8,
            "bq": np.ascontiguousarray(bq[cs]).reshape(2, 128),
            "bk": np.ascontiguousarray(8.0 * bk[cs]).reshape(2, 128),
            "bv": np.ascontiguousarray(bv[cs]),
            "xres": xres,
            "gamma": gamma,
            "beta": beta,
        }
        in_maps.append(m)

    nc = _get_program()
    trace_dir = os.environ.get("BASS_KERNEL_TRACE_DIR")
    kwargs = {}
    if trace_dir:
        kwargs = {"trace": True, "tmpdir": trace_dir}
    res = run_bass_kernel_spmd(nc, in_maps, core_ids=list(range(NCORES)), **kwargs)

    out = np.empty((B, S, D), np.float32)
    for c in range(NCORES):
        b, g = c // 4, c % 4
        yc = res.results[c]["y"]       # [QB, 128, D]
        for qb in range(QB):
            out[b, qb * 512 + g * 128: qb * 512 + (g + 1) * 128, :] = yc[qb]
    if trace_dir:
        kernel.last_exec_time_ns = res.exec_time_ns
        kernel.last_trace = res.instructions_and_trace
    return out


# revision 21
# speedup vs baseline: 1.6336x; 1.0171x over previous
"""Multi-head attention + residual + LayerNorm on 8 Trainium2 NeuronCores.

Reference computation (B=2, S=2048, D=1024, H=16, HD=64):
    q,k,v = split_heads(x@Wq+bq), ...       # [B,H,S,HD]
    attn  = softmax(q k^T / sqrt(HD))
    out   = (attn v) merged -> [B,S,D] @ Wp + bp
    y     = LayerNorm(x + out) * gamma + beta

Sharding: 8 cores = 2 batches x 4 head-groups (tensor parallel over the
16 heads).  Each core computes Q/K/V for its own 4 heads over the full
2048-token sequence (no K/V exchange needed), runs attention + the
partial out-projection for those heads, and the partials are
ReduceScattered (bf16, one op per 512-query block) across the 4 cores
of the batch.  Core g of a group owns query rows {qb*512 + g*128 ..
+128} for qb in 0..3 and finishes them with residual + LayerNorm.

Numerics: QKV projections and scores in bf16 (f32 PSUM accum); softmax
exp runs on the scalar engine directly from PSUM into fp8-e4m3
attention weights (with a uniform exp(-2) bias that cancels in the
normalization); V, the normalized attention output, and Wp are fp8 so
attn*V and the out-projection run in DoubleRow (dual-pumped fp8) mode.
Residual + LayerNorm in f32.  Validated vs the f32 reference:
rel err ~2e-3 (tolerance 2e-2).
"""

import os

import ml_dtypes
import numpy as np

import concourse.bacc as bacc
import concourse.bass as bass
import concourse.tile as tile
from concourse import mybir
from concourse.bass_utils import run_bass_kernel_spmd

B, S, D, H, HD = 2, 2048, 1024, 16, 64
EPS = 1e-5
NCORES = 8
GROUPS = [[0, 1, 2, 3], [4, 5, 6, 7]]
BF = mybir.dt.bfloat16
F8 = mybir.dt.float8e4
F32 = mybir.dt.float32
Act = mybir.ActivationFunctionType
Alu = mybir.AluOpType
DR = mybir.MatmulPerfMode.DoubleRow

QB = 4          # query blocks of 512
NH = 4          # heads per core
OSC = 32.0      # fp8 scale for normalized attn output
WSC = 16.0      # fp8 scale for Wp


def build_program():
    nc = bacc.Bacc("TRN2", target_bir_lowering=False, debug=False,
                   num_devices=NCORES)

    # ---- I/O ----
    xt_d = nc.dram_tensor("xt", [8, 128, S], BF, kind="ExternalInput")
    wq_d = nc.dram_tensor("wq", [8, 128, 256], BF, kind="ExternalInput")
    wk_d = nc.dram_tensor("wk", [8, 128, 256], BF, kind="ExternalInput")
    wv_d = nc.dram_tensor("wv", [8, 128, 256], BF, kind="ExternalInput")
    wp_d = nc.dram_tensor("wp", [64, 2, 2, D], F8, kind="ExternalInput")
    bq_d = nc.dram_tensor("bq", [2, 128], F32, kind="ExternalInput")
    bk_d = nc.dram_tensor("bk", [2, 128], F32, kind="ExternalInput")   # pre-scaled x8
    bv_d = nc.dram_tensor("bv", [256], F32, kind="ExternalInput")
    xres_d = nc.dram_tensor("xres", [QB, 128, D], F32, kind="ExternalInput")  # x rows + bp
    gamma_d = nc.dram_tensor("gamma", [D], F32, kind="ExternalInput")
    beta_d = nc.dram_tensor("beta", [D], F32, kind="ExternalInput")
    y_d = nc.dram_tensor("y", [QB, 128, D], F32, kind="ExternalOutput")
    # DRAM scratch for the per-unit reciprocal broadcast round-trip
    rdram = nc.dram_tensor("rdram", [16, 512], BF, kind="Internal")
    sdram = nc.dram_tensor("sdram", [16, 512], F32, kind="Internal")

    def bcast_ap(dram_t, n, parts=128):
        return bass.AP(tensor=dram_t, offset=0, ap=[[0, parts], [1, n]])

    def rd_ap(u, ap):
        return bass.AP(tensor=rdram, offset=u * 512, ap=ap)

    def sd_ap(u, ap):
        return bass.AP(tensor=sdram, offset=u * 512, ap=ap)

    with tile.TileContext(nc) as tc:
        with (
            tc.tile_pool(name="persist", bufs=1) as persist,
            tc.tile_pool(name="dram", bufs=1, space="DRAM") as dram,
        ):
            # persistent tiles
            xt_sb = persist.tile([128, 8, S], BF)            # x^T, D-chunk major
            wq_sb = persist.tile([128, 8, 256], BF)
            wk_sb = persist.tile([128, 8, 256], BF)
            wv_sb = persist.tile([128, 8, 256], BF)
            wp_sb = persist.tile([64, 2, 2, D], F8)
            kt_sb = [persist.tile([128, S], BF, name=f"kt{p}") for p in range(2)]
            qt_sb = [persist.tile([128, S], BF, name=f"qt{p}") for p in range(2)]
            vone = persist.tile([128, NH, 16, 80], F8)       # V | ones | pad (16B-aligned sub-stride)
            outT = persist.tile([64, NH, S], F8)             # normalized o^T * 32
            xres_sb = persist.tile([128, QB, D], F32)
            bq_sb = persist.tile([128, 2], F32)
            bk_sb = persist.tile([128, 2], F32)
            bv_bc = persist.tile([128, 4, 64], F32)
            gamma_bc = persist.tile([128, D], F32)
            beta_bc = persist.tile([128, D], F32)
            eps_sb = persist.tile([128, 1], F32)
            nb2_sb = persist.tile([128, 1], F32)     # exp bias: -2.0
            ones_sb = persist.tile([128, 64], BF)    # rank-1 broadcast row

            # DRAM scratch: ReduceScatter in/out per query block
            rs_in = [dram.tile([4, 128 * D], BF, name=f"rsi{i}") for i in range(QB)]
            rs_out = [dram.tile([128 * D], BF, name=f"rso{i}") for i in range(QB)]
            dumb_in = dram.tile([1, 128], BF, name="dumb_in")
            dumb_out = dram.tile([4, 128], BF, name="dumb_out")
            zrow = persist.tile([1, 128], BF)
            nc.vector.memset(zrow[:], 0.0)
            nc.gpsimd.dma_start(dumb_in[:], zrow[:])
            nc.gpsimd.collective_compute(
                "AllGather", Alu.bypass, replica_groups=GROUPS,
                ins=[dumb_in[:].opt()], outs=[dumb_out[:].opt()])

            # loads, in need-order; x^T lands in 512-token slices so the
            # K projection can start after the first ~1MB
            nc.sync.dma_start(wk_sb[:], wk_d.ap().rearrange("c p m -> p c m"))
            nc.sync.dma_start(bk_sb[:], bk_d.ap().rearrange("r p -> p r"))
            nc.sync.dma_start(wq_sb[:], wq_d.ap().rearrange("c p m -> p c m"))
            nc.sync.dma_start(bq_sb[:], bq_d.ap().rearrange("r p -> p r"))
            for s in range(4):
                nc.sync.dma_start(
                    xt_sb[:, :, s * 512:(s + 1) * 512],
                    xt_d.ap().rearrange("c p s -> p c s")[:, :, s * 512:(s + 1) * 512])
            nc.sync.dma_start(wv_sb[:], wv_d.ap().rearrange("c p m -> p c m"))
            nc.sync.dma_start(bv_bc[:], bcast_ap(bv_d, 256))
            nc.sync.dma_start(wp_sb[:], wp_d.ap())
            nc.sync.dma_start(xres_sb[:], xres_d.ap().rearrange("q p d -> p q d"))
            nc.sync.dma_start(gamma_bc[:], bcast_ap(gamma_d, D))
            nc.sync.dma_start(beta_bc[:], bcast_ap(beta_d, D))
            nc.vector.memset(eps_sb[:], EPS)
            nc.vector.memset(ones_sb[:], 1.0)
            nc.vector.memset(nb2_sb[:], -2.0)
            for h in range(NH):
                nc.vector.memset(vone[:, h, :, 64:80], 0.0)
                nc.vector.memset(vone[:, h, :, 64:65], 1.0)

            with (
                tc.tile_pool(name="work", bufs=2) as work,
                tc.tile_pool(name="expp", bufs=24) as expp,
                tc.tile_pool(name="small", bufs=4) as small,
                tc.tile_pool(name="ps_sc", bufs=2, space="PSUM") as ps_sc,
                tc.tile_pool(name="ps_o", bufs=2, space="PSUM") as ps_o,
            ):
                # ---------- phase 1 pieces ----------
                def qk_units(pair, typ, ncs):
                    # K^T / Q^T chunks for heads {2*pair, 2*pair+1}
                    if True:
                        w_sb = (wk_sb, wq_sb)[typ]
                        dst = (kt_sb, qt_sb)[typ][pair]
                        b_sb = (bk_sb, bq_sb)[typ]
                        for nck in ncs:
                            ps = ps_sc.tile([128, 2, 512], F32, tag="sc",
                                            name=f"qk{pair}{typ}{nck}")
                            for kc in range(8):
                                nc.tensor.matmul(
                                    ps[:, 0, :],
                                    w_sb[:, kc, pair * 128:(pair + 1) * 128],
                                    xt_sb[:, kc, nck * 512:(nck + 1) * 512],
                                    start=(kc == 0), stop=(kc == 7))
                            # (psum + bias) * scale  (K pre-scaled by 1/8)
                            with nc.allow_low_precision("K/Q in bf16"):
                                nc.vector.tensor_scalar(
                                    out=dst[:, nck * 512:(nck + 1) * 512],
                                    in0=ps[:, 0, :],
                                    scalar1=b_sb[:, pair:pair + 1],
                                    scalar2=0.125 if typ == 0 else 1.0,
                                    op0=Alu.add, op1=Alu.mult)

                def v_proj():
                    for tcn in range(16):
                        ps = ps_sc.tile([128, 2, 512], F32, tag="sc",
                                        name=f"v{tcn}")
                        for kc in range(8):
                            nc.tensor.matmul(
                                ps[:, 0, 0:256],
                                xt_sb[:, kc, tcn * 128:(tcn + 1) * 128],
                                wv_sb[:, kc, :],
                                start=(kc == 0), stop=(kc == 7))
                        with nc.allow_low_precision("attn V in fp8"):
                            nc.vector.tensor_add(
                                vone[:, :, tcn, 0:64],
                                ps[:, 0, 0:256].rearrange("p (h d) -> p h d", h=4),
                                bv_bc[:])

                # ---------- phase 2 pieces ----------
                def scores_exp(h, qb):
                    pair, half = h // 2, h % 2
                    r0 = half * 64
                    exp_ts = []
                    for j in range(8):      # key-chunk pairs of 256
                        ps = ps_sc.tile([128, 2, 512], F32, tag="sc",
                                        name=f"s{h}{qb}{j}")
                        for s in range(2):
                            kc = 2 * j + s
                            nc.tensor.matmul(
                                ps[:, s, :],
                                kt_sb[pair][r0:r0 + 64, kc * 128:(kc + 1) * 128],
                                qt_sb[pair][r0:r0 + 64, qb * 512:(qb + 1) * 512],
                                start=True, stop=True)
                        et = expp.tile([128, 2, 512], F8, tag="exp",
                                       name=f"e{h}{qb}{j}")
                        nc.scalar.activation(et[:], ps[:], Act.Exp, bias=nb2_sb[:])
                        exp_ts.append(et)
                    return exp_ts

                def attn_v(h, qb, exp_ts):
                    oT = ps_o.tile([80, 512], F32, tag="o", name=f"o{h}{qb}")
                    for j in range(8):
                        nc.tensor.matmul(
                            oT[:], vone[:, h, 2 * j:2 * j + 2, :], exp_ts[j][:],
                            start=(j == 0), stop=(j == 7), perf_mode=DR)
                    return oT

                def norm_o(h, qb, oT):
                    u = qb * NH + h
                    # sums row (partition 64) -> partition 0 via DMA, then
                    # reciprocal, rank-1 ones broadcast on the PE, and a
                    # staged scale+mult to fp8 (baseline-proven recipe)
                    s_sb = small.tile([128, 512], F32, tag="ssb", name=f"ss{u}")
                    nc.vector.tensor_copy(s_sb[64:65, :], oT[64:65, :])
                    s0 = small.tile([1, 512], F32, tag="s0", name=f"s0{u}")
                    nc.gpsimd.dma_start(s0[:], s_sb[64:65, :])
                    rr = small.tile([1, 512], F32, tag="rr", name=f"rw{u}")
                    nc.vector.reciprocal_approx_fast(out=rr[:], in_=s0[:])
                    rb = small.tile([1, 512], BF, tag="rb", name=f"rb{u}")
                    with nc.allow_low_precision("softmax scale bf16"):
                        nc.vector.tensor_copy(rb[:], rr[:])
                    r1 = ps_o.tile([128, 512], F32, tag="pj", name=f"r1{u}")
                    nc.tensor.matmul(r1[0:64, :], ones_sb[0:1, 0:64],
                                     rb[:], start=True, stop=True)
                    oSB = small.tile([64, 512], F32, tag="osb", name=f"ob{u}")
                    nc.vector.tensor_copy(oSB[:], oT[0:64, :])
                    with nc.allow_low_precision("attn out fp8"):
                        nc.vector.scalar_tensor_tensor(
                            out=outT[:, h, qb * 512:(qb + 1) * 512],
                            in0=oSB[:], scalar=OSC, in1=r1[0:64, :],
                            op0=Alu.mult, op1=Alu.mult)

                def unit(h, qb):
                    exp_ts = scores_exp(h, qb)
                    oT = attn_v(h, qb, exp_ts)
                    norm_o(h, qb, oT)

                def proj_rs(qb):
                    for qc in range(4):
                        yb = work.tile([128, 2, 512], BF, tag="yb", name=f"yb{qb}{qc}")
                        for ncn in range(2):
                            ps = ps_o.tile([128, 512], F32, tag="pj",
                                           name=f"pj{qb}{qc}{ncn}")
                            for c in range(2):
                                nc.tensor.matmul(
                                    ps[:],
                                    outT[:, 2 * c:2 * c + 2,
                                         qb * 512 + qc * 128:qb * 512 + (qc + 1) * 128],
                                    wp_sb[:, :, c, ncn * 512:(ncn + 1) * 512],
                                    start=(c == 0), stop=(c == 1), perf_mode=DR)
                            with nc.allow_low_precision("partial y bf16"):
                                nc.vector.tensor_scalar_mul(
                                    yb[:, ncn, :], ps[:], 1.0 / (OSC * WSC))
                        nc.gpsimd.dma_start(
                            rs_in[qb][qc].rearrange("(p d) -> p d", p=128), yb[:])
                    nc.gpsimd.collective_compute(
                        "ReduceScatter", Alu.add, replica_groups=GROUPS,
                        ins=[rs_in[qb][:].opt()], outs=[rs_out[qb][:].opt()])
                    # load of the scattered result (fires when the CC signals)
                    rsb = work.tile([128, D], BF, tag="rsb", name=f"rsb{qb}")
                    nc.gpsimd.dma_start(
                        rsb[:], rs_out[qb][:].rearrange("(p d) -> p d", p=128))
                    return rsb

                def post_pre(qb, rsb):
                    yt = work.tile([128, D], F32, tag="yt", name=f"yt{qb}",
                                   bufs=QB)
                    nc.vector.tensor_add(yt[:], rsb[:], xres_sb[:, qb, :])
                    stats = small.tile([128, 2, 6], F32, tag="st", name=f"st{qb}")
                    nc.vector.bn_stats(stats[:, 0, :], yt[:, 0:512])
                    nc.vector.bn_stats(stats[:, 1, :], yt[:, 512:1024])
                    mv = small.tile([128, 2], F32, tag="mv", name=f"mv{qb}",
                                    bufs=QB)
                    nc.vector.bn_aggr(mv[:], stats[:])
                    return yt, mv

                def post_fin(qb, yt, mv):
                    # ACT Ln/Exp deferred to the tail (under the last RS) so
                    # the activation-table swap never interrupts the exp
                    # stream.  rstd = exp(-0.5 * ln(var + eps)).
                    lnv = small.tile([128, 1], F32, tag="lnv", name=f"ln{qb}")
                    nc.scalar.activation(lnv[:], mv[:, 1:2], Act.Ln, bias=eps_sb[:])
                    rstd = small.tile([128, 1], F32, tag="rst", name=f"rs{qb}")
                    nc.scalar.activation(rstd[:], lnv[:], Act.Exp, scale=-0.5)
                    # y = ((y - mu) * gamma) * rstd + beta
                    nc.vector.scalar_tensor_tensor(
                        out=yt[:], in0=yt[:], scalar=mv[:, 0:1], in1=gamma_bc[:],
                        op0=Alu.subtract, op1=Alu.mult)
                    nc.vector.scalar_tensor_tensor(
                        out=yt[:], in0=yt[:], scalar=rstd[:], in1=beta_bc[:],
                        op0=Alu.mult, op1=Alu.add)
                    nc.sync.dma_start(y_d[qb], yt[:])

                # ---------- emission: software-pipelined ----------
                # proj_rs(qb) is deferred behind the first two units of
                # qb+1 so the PE queue never stalls on the softmax-
                # normalize chains; LN finalization happens in the tail
                # under the last ReduceScatter.
                qk_units(0, 0, [0, 1, 2, 3])     # K pair 0
                qk_units(0, 1, [0])              # Q pair 0, first 512 queries
                e0 = scores_exp(0, 0)
                qk_units(0, 1, [1, 2, 3])
                e1 = scores_exp(1, 0)
                v_proj()
                qk_units(1, 0, [0, 1, 2, 3])
                qk_units(1, 1, [0, 1, 2, 3])
                norm_o(0, 0, attn_v(0, 0, e0))
                norm_o(1, 0, attn_v(1, 0, e1))
                unit(2, 0)
                unit(3, 0)
                rsbs = {}
                posts = {}
                for qb in range(1, QB):
                    unit(0, qb)
                    rsbs[qb - 1] = proj_rs(qb - 1)
                    unit(1, qb)
                    unit(2, qb)
                    if qb >= 2:
                        posts[qb - 2] = post_pre(qb - 2, rsbs[qb - 2])
                    unit(3, qb)
                rsbs[QB - 1] = proj_rs(QB - 1)
                posts[QB - 2] = post_pre(QB - 2, rsbs[QB - 2])
                posts[QB - 1] = post_pre(QB - 1, rsbs[QB - 1])
                for qb in range(QB):
                    post_fin(qb, *posts[qb])

    nc.compile()
    return nc


_PROGRAM = None


def _get_program():
    global _PROGRAM
    if _PROGRAM is None:
        _PROGRAM = build_program()
    return _PROGRAM


def kernel(**inputs):
    x = np.asarray(inputs["x"], np.float32)
    bf = ml_dtypes.bfloat16
    f8 = ml_dtypes.float8_e4m3
    Wq = np.asarray(inputs["Wq"], np.float32)
    Wk = np.asarray(inputs["Wk"], np.float32)
    Wv = np.asarray(inputs["Wv"], np.float32)
    Wp = np.asarray(inputs["Wp"], np.float32)
    bq = np.asarray(inputs["bq"], np.float32)
    bk = np.asarray(inputs["bk"], np.float32)
    bv = np.asarray(inputs["bv"], np.float32)
    bp = np.asarray(inputs["bp"], np.float32)
    gamma = np.asarray(inputs["gamma"], np.float32)
    beta = np.asarray(inputs["beta"], np.float32)

    xt_b = []
    for b in range(B):
        xt = np.ascontiguousarray(x[b].T).astype(bf)          # [1024, 2048]
        xt_b.append(xt.reshape(8, 128, S))
    in_maps = []
    for c in range(NCORES):
        b, g = c // 4, c % 4
        cs = slice(g * 256, (g + 1) * 256)
        wp8 = (WSC * Wp[cs, :]).astype(f8).reshape(NH, 64, D)  # [head, d, D]
        wp8 = np.ascontiguousarray(
            wp8.reshape(2, 2, 64, D).transpose(2, 1, 0, 3))    # [64, sub, c, D]
        xres = np.empty((QB, 128, D), np.float32)
        for qb in range(QB):
            xres[qb] = x[b, qb * 512 + g * 128: qb * 512 + (g + 1) * 128, :] + bp
        m = {
            "xt": xt_b[b],
            "wq": np.ascontiguousarray(Wq[:, cs]).astype(bf).reshape(8, 128, 256),
            "wk": np.ascontiguousarray(Wk[:, cs]).astype(bf).reshape(8, 128, 256),
            "wv": np.ascontiguousarray(Wv[:, cs]).astype(bf).reshape(8, 128, 256),
            "wp": wp8,
            "bq": np.ascontiguousarray(bq[cs]).reshape(2, 128),
            "bk": np.ascontiguousarray(8.0 * bk[cs]).reshape(2, 128),
            "bv": np.ascontiguousarray(bv[cs]),
            "xres": xres,
            "gamma": gamma,
            "beta": beta,
        }
        in_maps.append(m)

    nc = _get_program()
    trace_dir = os.environ.get("BASS_KERNEL_TRACE_DIR")
    kwargs = {}
    if trace_dir:
        kwargs = {"trace": True, "tmpdir": trace_dir}
    res = run_bass_kernel_spmd(nc, in_maps, core_ids=list(range(NCORES)), **kwargs)

    out = np.empty((B, S, D), np.float32)
    for c in range(NCORES):
        b, g = c // 4, c % 4
        yc = res.results[c]["y"]       # [QB, 128, D]
        for qb in range(QB):
            out[b, qb * 512 + g * 128: qb * 512 + (g + 1) * 128, :] = yc[qb]
    if trace_dir:
        kernel.last_exec_time_ns = res.exec_time_ns
        kernel.last_trace = res.instructions_and_trace
    return out
